# revision 1
# baseline (speedup 1.0000x reference)
"""DiM block (adaLN MHA + adaLN MLP) Trainium2 Bass kernel.

Data-parallel over batch: B=8, one batch element per NeuronCore, weights
replicated, no collectives. Per core everything runs in a
feature-on-partition ("transposed") layout: the host pre-transposes x,
in_proj_w and out_proj_w (free), the kernel computes out.T and the host
transposes it back. Matmuls run in float32r (full PE rate, ~1.5e-4 rel
err; the PE rounds fp32 operands internally, so DRAM tensors are declared
float32r and loaded as plain HWDGE copies). LayerNorm statistics are
computed with all-ones matmuls (partition-dim reduction); modulation
vectors live as per-partition scalar columns produced by packed PE
transposes.

Self-contained: hardcodes all shapes; no sibling imports.
"""
import os
import sys

sys.path.insert(0, "/opt/trn_rl_repo")

import numpy as np

import concourse.bass as bass
import concourse.tile as tile
import concourse.mybir as mybir
from concourse import bacc
from concourse.bass_utils import run_bass_kernel_spmd
from concourse.masks import make_identity

D = 1024
N = 1024          # tokens per core
H = 8             # heads
DH = 128
DFF = 4096
KT = D // 128     # feature k-tiles
NT = N // 128     # token tiles
FT = DFF // 128   # mlp f-tiles
EPS = 1e-6
F32 = mybir.dt.float32
F32R = mybir.dt.float32r
AF = mybir.ActivationFunctionType
ALU = mybir.AluOpType

# rows tile / smalls column indices
R_IPBV, R_IPBQ, R_IPBK = 0, 1, 2
R_MG, R_MB, R_FG, R_FB = 3, 4, 5, 6
R_OPB, R_B2 = 7, 8
R_B1 = 9          # 9..12
R_C = 13
NROWS = 14
NROWS_M = 6       # mod rows tile: shift1,scale1,gate1,shift2,scale2,gate2
# smalls columns: 0..13 = rows, 14..19 = mod rows, 20..23 derived
R_SH1, R_SC1, R_G1, R_SH2, R_SC2, R_G2 = 14, 15, 16, 17, 18, 19
C_A1, C_C1, C_A2, C_C2 = 20, 21, 22, 23
NSMALL = 24


def f32(ap):
    return ap.bitcast(F32)


def _build():
    nc = bacc.Bacc("TRN2")

    xT_d = nc.dram_tensor("xT", [D, N], F32R, kind="ExternalInput")
    c = nc.dram_tensor("c", [1, D], F32R, kind="ExternalInput")
    m_ada_w = nc.dram_tensor("m_ada_w", [D, 3 * D], F32R, kind="ExternalInput")
    ada_b = nc.dram_tensor("ada_b", [1, 6 * D], F32, kind="ExternalInput")
    f_ada_w = nc.dram_tensor("f_ada_w", [D, 3 * D], F32R, kind="ExternalInput")
    norms = nc.dram_tensor("norms", [4, D], F32R, kind="ExternalInput")
    ipwT = nc.dram_tensor("ipwT", [D, 3 * D], F32R, kind="ExternalInput")
    in_proj_b = nc.dram_tensor("in_proj_b", [3, D], F32R, kind="ExternalInput")  # v,q,k
    opwT = nc.dram_tensor("opwT", [D, D], F32R, kind="ExternalInput")
    out_proj_b = nc.dram_tensor("out_proj_b", [1, D], F32R, kind="ExternalInput")
    w1 = nc.dram_tensor("w1", [D, DFF], F32R, kind="ExternalInput")
    b1 = nc.dram_tensor("b1", [4, D], F32R, kind="ExternalInput")
    w2 = nc.dram_tensor("w2", [DFF, D], F32R, kind="ExternalInput")
    b2 = nc.dram_tensor("b2", [1, D], F32R, kind="ExternalInput")
    outT = nc.dram_tensor("outT", [D, N], F32, kind="ExternalOutput")

    xT_r = xT_d.rearrange("(kt p) n -> p kt n", p=128)
    ipwT_r = ipwT.rearrange("(kt p) f -> p kt f", p=128)
    opwT_r = opwT.rearrange("(kt p) f -> p kt f", p=128)

    with tile.TileContext(nc) as tc, (
        tc.tile_pool(name="persist", bufs=1)
    ) as persist, tc.tile_pool(name="dram", bufs=1, space="DRAM") as dramp, (
        tc.tile_pool(name="ps1", bufs=4, space="PSUM")
    ) as ps1, tc.tile_pool(name="ps2", bufs=2, space="PSUM") as ps2, (
        tc.tile_pool(name="ph", bufs=1)
    ) as ph:

        ident = persist.tile([128, 128], F32)
        make_identity(nc, ident[:])
        ident_r = persist.tile([128, 128], F32R)
        nc.vector.tensor_copy(ident_r[:], ident[:])
        ones_f = persist.tile([128, 128], F32)
        nc.vector.memset(ones_f[:], 1.0)
        ones_r = persist.tile([128, 128], F32R)
        nc.vector.tensor_copy(ones_r[:], ones_f[:])
        eps_t = persist.tile([128, 1], F32)
        nc.vector.memset(eps_t[:], EPS)
        rows = persist.tile([NROWS, D], F32R)
        rows_m = persist.tile([NROWS_M, D], F32R)
        smalls = persist.tile([128, KT, NSMALL], F32R)

        def pe_transpose(dst_ap, src_ap, nr=128):
            """dst[128, nr] = src[nr, 128].T (both f32r)."""
            tp = ps1.tile([128, 512], F32, tag="ps1", name="tp")
            nc.tensor.matmul(
                tp[:, :nr].bitcast(F32R), src_ap, ident_r[:nr, :nr],
                is_transpose=True, start=True, stop=True,
            )
            nc.vector.tensor_copy(dst_ap, tp[:, :nr])

        # ---------------- phase 0a: vectors + silu(c) ------------------------
        nc.sync.dma_start(rows[R_IPBV : R_IPBV + 3, :], in_proj_b[:])
        nc.sync.dma_start(rows[R_MG : R_MG + 4, :], norms[:])
        nc.sync.dma_start(rows[R_OPB : R_OPB + 1, :], out_proj_b[:])
        nc.sync.dma_start(rows[R_B2 : R_B2 + 1, :], b2[:])
        nc.sync.dma_start(rows[R_B1 : R_B1 + 4, :], b1[:])
        c_sil = persist.tile([1, D], F32R, name="c_sil")
        nc.sync.dma_start(c_sil[:], c[:])
        nc.scalar.activation(c_sil[:], c_sil[:], AF.Silu)
        nc.sync.dma_start(rows[R_C : R_C + 1, :], c_sil[:])
        for kt in range(KT):
            pe_transpose(
                smalls[:, kt, :NROWS], rows[:, kt * 128 : (kt + 1) * 128], NROWS
            )

        # ---------------- layernorm helper (half-chunked) --------------------
        def layer_norm(srcT, dstT, ca, cc):
            with tc.tile_pool(name="pln", bufs=1) as pln:
                for ch in range(2):
                    sl = slice(ch * 512, (ch + 1) * 512)
                    s1 = ps2.tile([128, 512], F32, tag="ps2", name="s1")
                    s2 = ps2.tile([128, 512], F32, tag="ps2", name="s2")
                    for kt in range(KT):
                        nc.tensor.matmul(
                            s1[:], ones_r[:], srcT[:, kt, sl],
                            start=(kt == 0), stop=(kt == KT - 1),
                        )
                    for kt in range(KT):
                        xsq = pln.tile([128, 512], F32R, tag="xsq", bufs=2, name="xsq")
                        nc.vector.tensor_tensor(
                            xsq[:], f32(srcT[:, kt, sl]), f32(srcT[:, kt, sl]),
                            ALU.mult,
                        )
                        nc.tensor.matmul(
                            s2[:], ones_r[:], xsq[:],
                            start=(kt == 0), stop=(kt == KT - 1),
                        )
                    mu = pln.tile([128, 512], F32, tag="mu", bufs=2, name="mu")
                    var = pln.tile([128, 512], F32, tag="var", bufs=2, name="var")
                    rstd = pln.tile([128, 512], F32, tag="rstd", bufs=2, name="rstd")
                    nc.vector.tensor_scalar_mul(mu[:], s1[:], 1.0 / D)
                    nc.vector.tensor_scalar_mul(var[:], s2[:], 1.0 / D)
                    nc.vector.tensor_tensor(rstd[:], mu[:], mu[:], ALU.mult)
                    nc.vector.tensor_tensor(var[:], var[:], rstd[:], ALU.subtract)
                    nc.scalar.activation(var[:], var[:], AF.Sqrt, bias=eps_t[:])
                    nc.vector.reciprocal(rstd[:], var[:])
                    for kt in range(KT):
                        t1 = pln.tile([128, 512], F32, tag="lnt", bufs=2, name="t1")
                        nc.vector.tensor_tensor(
                            t1[:], f32(srcT[:, kt, sl]), mu[:], ALU.subtract
                        )
                        nc.vector.tensor_tensor(t1[:], t1[:], rstd[:], ALU.mult)
                        nc.vector.tensor_scalar(
                            dstT[:, kt, sl], t1[:],
                            f32(smalls[:, kt, ca : ca + 1]),
                            f32(smalls[:, kt, cc : cc + 1]),
                            ALU.mult, ALU.add,
                        )

        # ---------------- phase 1: xT load + mod + LN1 ------------------------
        x2d = dramp.tile([128, KT, N], F32, name="x2d")
        mod_stage = dramp.tile([NROWS_M, D], F32R, name="mod_stage")
        with tc.tile_pool(name="pxT1", bufs=1) as pxT1:
            xT = pxT1.tile([128, KT, N], F32R, name="xT")
            for kt in range(KT):
                nc.sync.dma_start(xT[:, kt, :], xT_r[:, kt, :])

            # ---- adaLN modulations: mod = silu(c) @ ada_w + ada_b ----------
            with tc.tile_pool(name="pmod", bufs=1) as pmod:
                adab_sb = pmod.tile([1, 6 * D], F32, name="adab_sb")
                nc.sync.dma_start(adab_sb[:], ada_b[:])
                for mi, aw in enumerate([m_ada_w, f_ada_w]):
                    aw_r = aw.rearrange("(kt p) f -> p kt f", p=128)
                    for ch in range(6):
                        sl = slice(ch * 512, (ch + 1) * 512)
                        wt = pmod.tile(
                            [128, KT, 512], F32R, tag="ada_w", bufs=2, name="wt"
                        )
                        nc.sync.dma_start(wt[:], aw_r[:, :, sl])
                        mp = ps1.tile([1, 512], F32, tag="ps1", name="mp")
                        for kt in range(KT):
                            nc.tensor.matmul(
                                mp[:], smalls[:, kt, R_C : R_C + 1], wt[:, kt, :],
                                start=(kt == 0), stop=(kt == KT - 1),
                            )
                        mb = pmod.tile([1, 512], F32R, tag="modbuf", bufs=2, name="mb")
                        nc.vector.tensor_tensor(
                            mb[:], mp[:],
                            adab_sb[:, mi * 3 * D + ch * 512 :][:, :512], ALU.add,
                        )
                        o0 = mi * 3 * D + ch * 512
                        r0, c0 = o0 // D, o0 % D
                        nc.sync.dma_start(
                            mod_stage[r0 : r0 + 1, c0 : c0 + 512], mb[:]
                        )
            nc.sync.dma_start(rows_m[:], mod_stage[:])
            for kt in range(KT):
                pe_transpose(
                    smalls[:, kt, R_SH1 : R_SH1 + NROWS_M],
                    rows_m[:, kt * 128 : (kt + 1) * 128],
                    NROWS_M,
                )
            # derived A/C columns: A = (1+scale)*g ; C = (1+scale)*b + shift
            with tc.tile_pool(name="pdrv", bufs=1) as pdrv:
                u = pdrv.tile([128, KT, 1], F32, name="u")
                for sc, sh, g_, b_, ca, cc in (
                    (R_SC1, R_SH1, R_MG, R_MB, C_A1, C_C1),
                    (R_SC2, R_SH2, R_FG, R_FB, C_A2, C_C2),
                ):
                    nc.vector.tensor_scalar_add(
                        u[:], f32(smalls[:, :, sc : sc + 1]), 1.0
                    )
                    nc.vector.tensor_tensor(
                        smalls[:, :, ca : ca + 1], u[:],
                        smalls[:, :, g_ : g_ + 1], ALU.mult,
                    )
                    nc.vector.tensor_tensor(
                        smalls[:, :, cc : cc + 1], u[:],
                        smalls[:, :, b_ : b_ + 1], ALU.mult,
                    )
                    nc.vector.tensor_tensor(
                        smalls[:, :, cc : cc + 1],
                        smalls[:, :, cc : cc + 1],
                        smalls[:, :, sh : sh + 1], ALU.add,
                    )

            hT = ph.tile([128, KT, N], F32R, tag="hT", name="h1T")
            layer_norm(xT, hT, C_A1, C_C1)

        inv_sqrt_dh = float(1.0 / np.sqrt(DH))
        with tc.tile_pool(name="po", bufs=1) as po:
            oT_all = po.tile([128, H, N], F32R, name="oT_all")
            with tc.tile_pool(name="pv", bufs=1) as pv:
                # ------------ phase 2: v_nat ---------------------------------
                v_nat = pv.tile([128, NT, D], F32R, name="v_nat")
                with tc.tile_pool(name="pwv", bufs=1) as pwv:
                    wvT = pwv.tile([128, KT, D], F32R, name="wvT")
                    nc.sync.dma_start(wvT[:], ipwT_r[:, :, 2 * D : 3 * D])
                    for nt in range(NT):
                        for ch in range(2):
                            sl = slice(ch * 512, (ch + 1) * 512)
                            vp = ps1.tile([128, 512], F32, tag="ps1", name="vp")
                            for kt in range(KT):
                                nc.tensor.matmul(
                                    vp[:], hT[:, kt, nt * 128 : (nt + 1) * 128],
                                    wvT[:, kt, sl], start=(kt == 0), stop=False,
                                )
                            nc.tensor.matmul(
                                vp[:], ones_r[0:1, :],
                                rows[R_IPBV : R_IPBV + 1, sl],
                                start=False, stop=True,
                            )
                            nc.vector.tensor_copy(v_nat[:, nt, sl], vp[:])

                # ------------ phase 3: attention --------------------------
                with tc.tile_pool(name="pattn", bufs=1) as pa:
                    for h in range(H):
                        wqT = pa.tile([128, KT, 128], F32R, tag="wqT", bufs=2,
                                      name="wqT")
                        wkT = pa.tile([128, KT, 128], F32R, tag="wkT", bufs=2,
                                      name="wkT")
                        nc.sync.dma_start(
                            wqT[:], ipwT_r[:, :, h * 128 : (h + 1) * 128]
                        )
                        nc.sync.dma_start(
                            wkT[:], ipwT_r[:, :, D + h * 128 : D + (h + 1) * 128]
                        )
                        qT = pa.tile([128, N], F32R, tag="qT", name="qT")
                        kTt = pa.tile([128, N], F32R, tag="kTt", name="kTt")
                        for ch in range(2):
                            sl = slice(ch * 512, (ch + 1) * 512)
                            for dst, wT, brow in (
                                (qT, wqT, R_IPBQ), (kTt, wkT, R_IPBK)
                            ):
                                pp = ps1.tile([128, 512], F32, tag="ps1", name="pp")
                                for kt in range(KT):
                                    nc.tensor.matmul(
                                        pp[:], wT[:, kt, :], hT[:, kt, sl],
                                        start=(kt == 0), stop=(kt == KT - 1),
                                    )
                                nc.vector.tensor_scalar(
                                    dst[:, sl], pp[:],
                                    f32(smalls[:, h, brow : brow + 1]),
                                    None, ALU.add,
                                )
                        for qh in range(2):
                            qsl = slice(qh * 512, (qh + 1) * 512)
                            expT = pa.tile(
                                [128, KT, 512], F32R, tag="expT", bufs=2, name="expT"
                            )
                            for kt in range(KT):
                                sp = ps1.tile([128, 512], F32, tag="ps1", name="sp")
                                nc.tensor.matmul(
                                    sp[:], kTt[:, kt * 128 : (kt + 1) * 128],
                                    qT[:, qsl], start=True, stop=True,
                                )
                                nc.scalar.activation(
                                    expT[:, kt, :], sp[:], AF.Exp,
                                    scale=inv_sqrt_dh,
                                )
                            lb = ps1.tile([128, 512], F32, tag="ps1", name="lb")
                            for kt in range(KT):
                                nc.tensor.matmul(
                                    lb[:], ones_r[:], expT[:, kt, :],
                                    start=(kt == 0), stop=(kt == KT - 1),
                                )
                            linv = pa.tile(
                                [128, 512], F32, tag="linv", bufs=2, name="linv"
                            )
                            nc.vector.reciprocal(linv[:], lb[:])
                            op = ps1.tile([128, 512], F32, tag="ps1", name="op")
                            for kt in range(KT):
                                nc.tensor.matmul(
                                    op[:], v_nat[:, kt, h * 128 : (h + 1) * 128],
                                    expT[:, kt, :],
                                    start=(kt == 0), stop=(kt == KT - 1),
                                )
                            nc.vector.tensor_tensor(
                                oT_all[:, h, qsl], op[:], linv[:], ALU.mult
                            )

                # ------------ phase 4: out_proj + residual 1 + LN2 ---------
                # reuse v_nat's slot: reload starts once head-7's output
                # matmuls release v_nat (before attention fully drains)
                xT2 = pv.tile([128, KT, N], F32R, tag="v_nat", name="xT2")
                for kt in range(KT):
                    nc.sync.dma_start(xT2[:, kt, :], xT_r[:, kt, :])
                with tc.tile_pool(name="pwo", bufs=1) as pwo:
                    # reuse the (dead) h1T slot: the load starts as soon as
                    # the last head's q/k projections release h1T
                    woT = ph.tile([128, KT, D], F32R, tag="hT", name="woT")
                    nc.sync.dma_start(woT[:], opwT_r[:])
                    for dt_ in range(KT):
                        pp = ps2.tile([128, N], F32, tag="ps2", name="pp2")
                        for ch in range(2):
                            sl = slice(ch * 512, (ch + 1) * 512)
                            for kt in range(KT):
                                nc.tensor.matmul(
                                    pp[:, sl],
                                    woT[:, kt, dt_ * 128 : (dt_ + 1) * 128],
                                    oT_all[:, kt, sl],
                                    start=(kt == 0), stop=(kt == KT - 1),
                                )
                        t = pwo.tile([128, N], F32, tag="res1", name="res1")
                        nc.vector.tensor_scalar(
                            t[:], pp[:],
                            f32(smalls[:, dt_, R_OPB : R_OPB + 1]),
                            f32(smalls[:, dt_, R_G1 : R_G1 + 1]),
                            ALU.add, ALU.mult,
                        )
                        nc.vector.tensor_tensor(
                            xT2[:, dt_, :], t[:], f32(xT2[:, dt_, :]), ALU.add
                        )

                nc.sync.dma_start(x2d[:], f32(xT2[:]))
                h2T = ph.tile([128, KT, N], F32R, tag="hT", name="h2T")
                layer_norm(xT2, h2T, C_A2, C_C2)

        # ---------------- phase 5: MLP + residual 2 + out --------------------
        w1_r = w1.rearrange("(kt p) f -> p kt f", p=128)
        w2_r = w2.rearrange("(ft p) d -> p ft d", p=128)
        with tc.tile_pool(name="pmlp", bufs=1) as pm:
            for hh in range(2):
                tsl = slice(hh * 512, (hh + 1) * 512)
                gT = pm.tile([128, FT, 512], F32R, tag="gT", name="gT")
                for ft in range(FT):
                    w1t = pm.tile(
                        [128, KT, 128], F32R, tag="w1t", bufs=3, name="w1t"
                    )
                    nc.sync.dma_start(
                        w1t[:], w1_r[:, :, ft * 128 : (ft + 1) * 128]
                    )
                    gp = ps1.tile([128, 512], F32, tag="ps1", name="gp")
                    for kt in range(KT):
                        nc.tensor.matmul(
                            gp[:], w1t[:, kt, :], h2T[:, kt, tsl],
                            start=(kt == 0), stop=(kt == KT - 1),
                        )
                    nc.scalar.activation(
                        gT[:, ft, :], gp[:], AF.Gelu,
                        bias=f32(
                            smalls[:, ft % 8, R_B1 + ft // 8 : R_B1 + ft // 8 + 1]
                        ),
                    )
                out2h = pm.tile([128, KT, 512], F32R, tag="out2h", name="out2h")
                for dt_ in range(KT):
                    yp = ps1.tile([128, 512], F32, tag="ps1", name="yp")
                    for fh in range(2):
                        w2t = pm.tile(
                            [128, 16, 128], F32R, tag="w2t", bufs=3, name="w2t"
                        )
                        nc.sync.dma_start(
                            w2t[:],
                            w2_r[
                                :, fh * 16 : (fh + 1) * 16,
                                dt_ * 128 : (dt_ + 1) * 128,
                            ],
                        )
                        for j in range(16):
                            ft = fh * 16 + j
                            nc.tensor.matmul(
                                yp[:], w2t[:, j, :], gT[:, ft, :],
                                start=(ft == 0), stop=(ft == FT - 1),
                            )
                    nc.vector.tensor_scalar(
                        out2h[:, dt_, :], yp[:],
                        f32(smalls[:, dt_, R_B2 : R_B2 + 1]),
                        f32(smalls[:, dt_, R_G2 : R_G2 + 1]),
                        ALU.add, ALU.mult,
                    )
                    # accumulate residual stream per d-tile, then store outT
                    nc.gpsimd.dma_start(
                        out2h[:, dt_, :], x2d[:, dt_, tsl].bitcast(F32R),
                        accum_op=ALU.add,
                    )
                    nc.sync.dma_start(
                        outT[dt_ * 128 : (dt_ + 1) * 128, tsl],
                        f32(out2h[:, dt_, :]),
                    )

    nc.compile()
    return nc


_NC_CACHE = None


def _get_nc():
    global _NC_CACHE
    if _NC_CACHE is None:
        _NC_CACHE = _build()
    return _NC_CACHE


def kernel(**inputs):
    B = 8
    f = lambda a: np.ascontiguousarray(np.asarray(a), dtype=np.float32)
    ipb = f(inputs["in_proj_b"]).reshape(3, D)  # q,k,v rows
    shared = {
        "m_ada_w": f(inputs["m_ada_w"]),
        "f_ada_w": f(inputs["f_ada_w"]),
        "ada_b": np.concatenate(
            [f(inputs["m_ada_b"]).reshape(-1), f(inputs["f_ada_b"]).reshape(-1)]
        ).reshape(1, 6 * D),
        "norms": np.stack(
            [
                f(inputs["m_norm_g"]).reshape(-1),
                f(inputs["m_norm_b"]).reshape(-1),
                f(inputs["f_norm_g"]).reshape(-1),
                f(inputs["f_norm_b"]).reshape(-1),
            ]
        ),
        "ipwT": np.ascontiguousarray(f(inputs["in_proj_w"]).T),
        "in_proj_b": np.ascontiguousarray(ipb[[2, 0, 1]]),  # v,q,k
        "opwT": np.ascontiguousarray(f(inputs["out_proj_w"]).T),
        "out_proj_b": f(inputs["out_proj_b"]).reshape(1, D),
        "w1": f(inputs["w1"]),
        "b1": f(inputs["b1"]).reshape(4, D),
        "w2": f(inputs["w2"]),
        "b2": f(inputs["b2"]).reshape(1, D),
    }
    x = f(inputs["x"])
    c = f(inputs["c"])
    in_maps = [
        {
            "xT": np.ascontiguousarray(x[b].T),
            "c": np.ascontiguousarray(c[b : b + 1]),
            **shared,
        }
        for b in range(B)
    ]
    nc = _get_nc()
    br = run_bass_kernel_spmd(nc, in_maps, core_ids=list(range(B)))
    o = np.stack([r["outT"] for r in br.results])  # [B, D, N]
    return np.ascontiguousarray(o.transpose(0, 2, 1)).astype(np.float32)

